# revision 5
# baseline (speedup 1.0000x reference)
"""Trainium2 Bass kernel for nn_Algebraic_65970697666729 (segment_reduce).

Computes, for x of shape (131072, 16) fp32:
    out = concat([x, all C(16,2)=120 pairwise products, all C(16,3)=560
                  triple products], axis=1)  -> (131072, 696) fp32

Sharding: pure data parallel over rows; 8 cores x 16384 rows each.

Per-core layout: partition p holds 128 consecutive rows (row = p*128 + r),
so the input load is one fully-contiguous 1MB DMA and every output store is
contiguous per partition (R*2784B runs).

Compute (per row tile, all on the vector engine, fp32):
  - pairs:   for i in 0..14:  out[16+po(i) : ...] = bcast(x_i) * x[i+1:16]
  - triples: for i in 0..13:  triples with first index i are exactly
             bcast(x_i) * (pairs whose first index >= i+1), which is a
             contiguous tail of the pairs section just computed.
That is 29 tensor_mul instructions per tile, one multiply per output
element, plus one scalar-engine copy for the leading x section.
"""

import numpy as np

N_CORES = 8
ROWS_TOTAL = 131072
ROWS = ROWS_TOTAL // N_CORES  # 16384
N = 16
NPAIRS = 120
NTRIPLES = 560
OUT = N + NPAIRS + NTRIPLES  # 696
P = 128

# Row-tile schedule (rows-per-partition per tile); sums to ROWS // P == 128.
# Small leading tiles get the output DMA pipeline started early.
R_SCHEDULE = [4, 6, 8, 12, 16, 24, 32, 26]

# Triple runs (by first index i) computed on GpSimd instead of the vector
# engine. Empty: GpSimd shares SBUF ports with the vector engine, and
# running both concurrently slowed DVE tensor_tensor by ~2.4x per element
# (measured), a large net loss.
POOL_TRIPLES: set = set()

_CACHE = {}


def _pair_offsets():
    # po[i] = index (within the pairs section) of the first pair (i, *)
    po = [0] * (N + 1)
    for i in range(1, N + 1):
        po[i] = po[i - 1] + (N - 1 - (i - 1))
    return po


def _triple_offsets():
    # to[i] = index (within the triples section) of the first triple (i, *, *)
    to = [0] * N
    for i in range(1, N):
        m = N - 1 - (i - 1)  # suffix size after index i-1
        to[i] = to[i - 1] + m * (m - 1) // 2
    return to


def _build():
    import concourse.bacc as bacc
    import concourse.mybir as mybir
    from concourse import tile

    f32 = mybir.dt.float32
    nc = bacc.Bacc(
        "TRN2",
        target_bir_lowering=False,
        debug=False,
        enable_asserts=True,
        num_devices=N_CORES,
    )
    x = nc.dram_tensor("x", [ROWS, N], f32, kind="ExternalInput")
    out = nc.dram_tensor("out", [ROWS, OUT], f32, kind="ExternalOutput")
    xv = x.ap().rearrange("(p r) f -> p r f", p=P)  # [128, 128, 16]
    ov = out.ap().rearrange("(p r) c -> p r c", p=P)  # [128, 128, 696]

    po = _pair_offsets()
    to = _triple_offsets()

    with tile.TileContext(nc) as tc:
        with (
            tc.tile_pool(name="xp", bufs=1) as xp,
            tc.tile_pool(name="op", bufs=2) as op,
        ):
            xt = xp.tile([P, ROWS // P, N], f32)
            # Split the input load so the first (small) tile's compute can
            # start without waiting for the whole 1MB.
            R0 = R_SCHEDULE[0]
            nc.sync.dma_start(out=xt[:, 0:R0, :], in_=xv[:, 0:R0, :])
            nc.sync.dma_start(
                out=xt[:, R0 : ROWS // P, :], in_=xv[:, R0 : ROWS // P, :]
            )

            r0 = 0
            for R in R_SCHEDULE:
                ot = op.tile([P, R, OUT], f32, tag="out")
                xs = xt[:, r0 : r0 + R, :]

                nc.scalar.copy(out=ot[:, :, 0:N], in_=xs)

                for i in range(N - 1):
                    L = N - 1 - i
                    a = N + po[i]
                    nc.vector.tensor_mul(
                        out=ot[:, :, a : a + L],
                        in0=xs[:, :, i + 1 : N],
                        in1=xs[:, :, i : i + 1].broadcast_to([P, R, L]),
                    )

                for i in range(N - 2):
                    m = N - 1 - i  # suffix size after i
                    L = m * (m - 1) // 2
                    a = N + NPAIRS + to[i]
                    # The largest triple runs go to the (otherwise idle)
                    # GpSimd engine; nothing reads the triples, so this
                    # costs only a pairs->GpSimd dependency.
                    eng = nc.gpsimd if i in POOL_TRIPLES else nc.vector
                    eng.tensor_mul(
                        out=ot[:, :, a : a + L],
                        in0=ot[:, :, N + po[i + 1] : N + NPAIRS],
                        in1=xs[:, :, i : i + 1].broadcast_to([P, R, L]),
                    )

                nc.sync.dma_start(out=ov[:, r0 : r0 + R, :], in_=ot[:])
                r0 += R
            assert r0 == ROWS // P

    nc.compile()
    return nc


def _run(x, trace=False, **spmd_kwargs):
    from concourse.bass_utils import run_bass_kernel_spmd

    if "nc" not in _CACHE:
        _CACHE["nc"] = _build()
    nc = _CACHE["nc"]

    x = np.ascontiguousarray(np.asarray(x, dtype=np.float32))
    assert x.shape == (ROWS_TOTAL, N), x.shape
    chunks = x.reshape(N_CORES, ROWS, N)
    in_maps = [{"x": np.ascontiguousarray(chunks[i])} for i in range(N_CORES)]
    res = run_bass_kernel_spmd(
        nc, in_maps, core_ids=list(range(N_CORES)), trace=trace, **spmd_kwargs
    )
    full = np.concatenate([r["out"] for r in res.results], axis=0)
    return full, res


def kernel(x):
    return _run(x)[0]


# revision 8
# speedup vs baseline: 1.1046x; 1.1046x over previous
"""Trainium2 Bass kernel for nn_Algebraic_65970697666729 (segment_reduce).

Computes, for x of shape (131072, 16) fp32:
    out = concat([x, all C(16,2)=120 pairwise products, all C(16,3)=560
                  triple products], axis=1)  -> (131072, 696) fp32

Sharding: pure data parallel over rows; 8 cores x 16384 rows each.

Per-core layout: partition p holds 128 consecutive rows (row = p*128 + r),
so the input load is one fully-contiguous 1MB DMA and every output store is
contiguous per partition (R*2784B runs).

Compute (per row tile, all on the vector engine, fp32):
  - pairs:   for i in 0..14:  out[16+po(i) : ...] = bcast(x_i) * x[i+1:16]
  - triples: for i in 0..13:  triples with first index i are exactly
             bcast(x_i) * (pairs whose first index >= i+1), which is a
             contiguous tail of the pairs section just computed.
That is 29 tensor_mul instructions per tile, one multiply per output
element, plus one scalar-engine copy for the leading x section.
"""

import numpy as np

N_CORES = 8
ROWS_TOTAL = 131072
ROWS = ROWS_TOTAL // N_CORES  # 16384
N = 16
NPAIRS = 120
NTRIPLES = 560
OUT = N + NPAIRS + NTRIPLES  # 696
P = 128

# Row-tile schedule (rows-per-partition per tile); sums to ROWS // P == 128.
# Small leading tiles get the output DMA pipeline started early.
R_SCHEDULE = [8, 24, 32, 32, 32]

# Ship each tile's output in two DMAs: the x+pairs section (cols 0:136) as
# soon as the pairs are done, the triples section (cols 136:696) after the
# rest. This keeps the DMA stream fed while triples are still computing.
SPLIT_SECTIONS = True

# Triple runs (by first index i) computed on GpSimd instead of the vector
# engine. Empty: GpSimd shares SBUF ports with the vector engine, and
# running both concurrently slowed DVE tensor_tensor by ~2.4x per element
# (measured), a large net loss.
POOL_TRIPLES: set = set()

_CACHE = {}


def _pair_offsets():
    # po[i] = index (within the pairs section) of the first pair (i, *)
    po = [0] * (N + 1)
    for i in range(1, N + 1):
        po[i] = po[i - 1] + (N - 1 - (i - 1))
    return po


def _triple_offsets():
    # to[i] = index (within the triples section) of the first triple (i, *, *)
    to = [0] * N
    for i in range(1, N):
        m = N - 1 - (i - 1)  # suffix size after index i-1
        to[i] = to[i - 1] + m * (m - 1) // 2
    return to


def _build():
    import concourse.bacc as bacc
    import concourse.mybir as mybir
    from concourse import tile

    f32 = mybir.dt.float32
    nc = bacc.Bacc(
        "TRN2",
        target_bir_lowering=False,
        debug=False,
        enable_asserts=True,
        num_devices=N_CORES,
    )
    x = nc.dram_tensor("x", [ROWS, N], f32, kind="ExternalInput")
    out = nc.dram_tensor("out", [ROWS, OUT], f32, kind="ExternalOutput")
    xv = x.ap().rearrange("(p r) f -> p r f", p=P)  # [128, 128, 16]
    ov = out.ap().rearrange("(p r) c -> p r c", p=P)  # [128, 128, 696]

    po = _pair_offsets()
    to = _triple_offsets()

    with tile.TileContext(nc) as tc:
        with (
            tc.tile_pool(name="xp", bufs=1) as xp,
            tc.tile_pool(name="op", bufs=2) as op,
        ):
            xt = xp.tile([P, ROWS // P, N], f32)
            # Split the input load so the first (small) tile's compute can
            # start without waiting for the whole 1MB.
            R0 = R_SCHEDULE[0]
            nc.sync.dma_start(out=xt[:, 0:R0, :], in_=xv[:, 0:R0, :])
            nc.sync.dma_start(
                out=xt[:, R0 : ROWS // P, :], in_=xv[:, R0 : ROWS // P, :]
            )

            r0 = 0
            for R in R_SCHEDULE:
                ot = op.tile([P, R, OUT], f32, tag="out")
                xs = xt[:, r0 : r0 + R, :]

                nc.scalar.copy(out=ot[:, :, 0:N], in_=xs)

                for i in range(N - 1):
                    L = N - 1 - i
                    a = N + po[i]
                    nc.vector.tensor_mul(
                        out=ot[:, :, a : a + L],
                        in0=xs[:, :, i + 1 : N],
                        in1=xs[:, :, i : i + 1].broadcast_to([P, R, L]),
                    )

                if SPLIT_SECTIONS:
                    nc.sync.dma_start(
                        out=ov[:, r0 : r0 + R, 0 : N + NPAIRS],
                        in_=ot[:, :, 0 : N + NPAIRS],
                    )

                for i in range(N - 2):
                    m = N - 1 - i  # suffix size after i
                    L = m * (m - 1) // 2
                    a = N + NPAIRS + to[i]
                    # The largest triple runs go to the (otherwise idle)
                    # GpSimd engine; nothing reads the triples, so this
                    # costs only a pairs->GpSimd dependency.
                    eng = nc.gpsimd if i in POOL_TRIPLES else nc.vector
                    eng.tensor_mul(
                        out=ot[:, :, a : a + L],
                        in0=ot[:, :, N + po[i + 1] : N + NPAIRS],
                        in1=xs[:, :, i : i + 1].broadcast_to([P, R, L]),
                    )

                if SPLIT_SECTIONS:
                    nc.sync.dma_start(
                        out=ov[:, r0 : r0 + R, N + NPAIRS : OUT],
                        in_=ot[:, :, N + NPAIRS : OUT],
                    )
                else:
                    nc.sync.dma_start(out=ov[:, r0 : r0 + R, :], in_=ot[:])
                r0 += R
            assert r0 == ROWS // P

    nc.compile()
    return nc


def _run(x, trace=False, **spmd_kwargs):
    from concourse.bass_utils import run_bass_kernel_spmd

    if "nc" not in _CACHE:
        _CACHE["nc"] = _build()
    nc = _CACHE["nc"]

    x = np.ascontiguousarray(np.asarray(x, dtype=np.float32))
    assert x.shape == (ROWS_TOTAL, N), x.shape
    chunks = x.reshape(N_CORES, ROWS, N)
    in_maps = [{"x": np.ascontiguousarray(chunks[i])} for i in range(N_CORES)]
    res = run_bass_kernel_spmd(
        nc, in_maps, core_ids=list(range(N_CORES)), trace=trace, **spmd_kwargs
    )
    full = np.concatenate([r["out"] for r in res.results], axis=0)
    return full, res


def kernel(x):
    return _run(x)[0]
